# revision 78
# baseline (speedup 1.0000x reference)
"""HG-GNN fused Bass kernel for 8 Trainium2 NeuronCores (v3).

Sharding:
  - Phases 1-4 (SAGE + session attention) are batch-parallel: 128 sessions
    per core; each core computes SAGE h1 rows only for the 2688 node
    positions its batch references, with exact full in-edge lists (host
    CSR-ifies by dst).
  - Phase 5 (scoring) is item-parallel: session vectors seqT [128f,128s]
    are AllGathered across the 8 cores (DRAM collective), then each core
    scores ALL 1024 sessions against its own 6250-item column slice of the
    node table (streamed during the scoring phase, overlapped with the
    score writeback). Host assembles [1024, 50000] from column slices.
  - fp16 everywhere on device (PSUM fp32); host upcasts scores.
  - pos/self embeddings arrive via transpose-mode dma_gather directly in
    [feature, slot] layout (pos table duplicated to 256-wide rows; node
    table viewed as pairs with an on-chip even/odd select).
"""

import os
import sys

import numpy as np

if "/opt/trn_rl_repo" not in sys.path:
    sys.path.insert(0, "/opt/trn_rl_repo")

import concourse.bass as bass
import concourse.tile as tile
from concourse import bacc, mybir
from concourse.bass_utils import run_bass_kernel_spmd

ITEM_NUM = 40000
NUM_USERS = 10000
NN = ITEM_NUM + NUM_USERS  # 50000
EM = 128
BS = 1024
L = 20
POSN = 200
NCORES = 8
SB = BS // NCORES  # 128 sessions per core
NSL = NN // NCORES  # 6250 item-slice per core for scoring
T = SB * L  # 2560 item tokens per core
NWIN = L + 1  # 20 item windows + 1 user window (128 positions each)
NPOS = NWIN * 128  # 2688 positions per core
ATT_CHUNK = 512
SCORE_CHUNK = 2048
GW = 1  # windows per edge-gather group

F32 = mybir.dt.float32
F16 = mybir.dt.float16
I32 = mybir.dt.int32
I16 = mybir.dt.int16

_CACHE: dict = {}


# --------------------------------------------------------------------------
# host-side preprocessing
# --------------------------------------------------------------------------


def wrap16(flat):
    # dma_gather index layout: [16, n/16] wrap, replicated to 128 partitions
    n = len(flat)
    wrapped = np.zeros((16, (n + 15) // 16), np.int16)
    wrapped[np.arange(n) % 16, np.arange(n) // 16] = flat
    return np.tile(wrapped, (8, 1))


def _preprocess(src, dst, user, seq, mask, pos_idx):
    """Build per-core device arrays. Returns (per_core list of dicts, CH)."""
    src = np.asarray(src).astype(np.int64)
    dst = np.asarray(dst).astype(np.int64)
    user = np.asarray(user).astype(np.int64)
    seq = np.asarray(seq).astype(np.int64)
    mask = np.asarray(mask).astype(np.float32)
    pos_idx = np.asarray(pos_idx).astype(np.int64)

    order = np.argsort(dst, kind="stable")
    src_sorted = src[order].astype(np.int32)
    deg = np.bincount(dst, minlength=NN).astype(np.int64)
    row_ptr = np.zeros(NN + 1, dtype=np.int64)
    np.cumsum(deg, out=row_ptr[1:])
    inv_deg = (1.0 / np.maximum(deg, 1)).astype(np.float32)

    cores = []
    SPLIT = 32768
    nodes_all = []
    for c in range(NCORES):
        seq_c = seq[c * SB : (c + 1) * SB]  # [128, 20]
        user_c = user[c * SB : (c + 1) * SB] + ITEM_NUM  # [128]
        pos_nodes = np.concatenate([seq_c.reshape(-1), user_c])  # [2688]
        nodes_all.append(pos_nodes)

    lists = [[None] * NWIN for _ in range(NCORES)]
    for c in range(NCORES):
        for w in range(NWIN):
            nodes_w = nodes_all[c][w * 128 : (w + 1) * 128]
            cnt = deg[nodes_w]
            Lw = int(cnt.sum())
            if Lw > 0:
                starts = row_ptr[nodes_w]
                ends = np.cumsum(cnt)
                offs = np.arange(Lw, dtype=np.int64) - np.repeat(ends - cnt, cnt)
                gidx = np.repeat(starts, cnt) + offs
                srcs = src_sorted[gidx]
                dl = np.repeat(np.arange(128), cnt)
            else:
                srcs = np.zeros(0, np.int32)
                dl = np.zeros(0, np.int64)
            selA = srcs < SPLIT
            lists[c][w] = (
                srcs[selA],
                dl[selA],
                srcs[~selA] - SPLIT,
                dl[~selA],
            )

    CHA, CHB = [], []
    for w in range(NWIN):
        mA = mB = 0
        for c in range(NCORES):
            sA, _, sB, _ = lists[c][w]
            mA = max(mA, (len(sA) + 127) // 128)
            mB = max(mB, (len(sB) + 127) // 128)
        CHA.append(mA)
        CHB.append(mB)
    CH = (tuple(CHA), tuple(CHB))
    CTA, CTB = int(np.sum(CHA)), int(np.sum(CHB))
    csA = np.concatenate([[0], np.cumsum(CHA)]).astype(np.int64)
    csB = np.concatenate([[0], np.cumsum(CHB)]).astype(np.int64)

    # combined per-window dstloc: A columns then B columns
    csAB = csA + csB

    for c in range(NCORES):
        pos_nodes = nodes_all[c]
        idxA = np.zeros((128, 8 * CTA), dtype=np.int16)
        idxB = np.zeros((128, 8 * max(CTB, 1)), dtype=np.int16)
        dstlocAB = np.full((128, CTA + CTB), -1.0, dtype=np.float16)
        for w in range(NWIN):
            sA, dA, sB, dB = lists[c][w]
            for (s_, d_, chn, cs_, idx_, dcol) in (
                (sA, dA, CHA[w], csA[w], idxA, int(csAB[w])),
                (sB, dB, CHB[w], csB[w], idxB, int(csAB[w]) + CHA[w]),
            ):
                if chn == 0:
                    continue
                cap = chn * 128
                flat = np.zeros(cap, np.int16)
                flat[: len(s_)] = s_.astype(np.int16)
                dl = np.full(cap, -1.0, np.float32)
                dl[: len(d_)] = d_.astype(np.float32)
                idx_[:, 8 * cs_ : 8 * (cs_ + chn)] = wrap16(flat)
                dstlocAB[:, dcol : dcol + chn] = (
                    dl.reshape(chn, 128).T.astype(np.float16)
                )

        invdeg_pos = inv_deg[pos_nodes]  # [2688]
        invdegb = np.ascontiguousarray(
            np.broadcast_to(invdeg_pos[None, :], (128, NPOS))
        ).astype(np.float16)

        mask_c = mask[c * SB : (c + 1) * SB].reshape(-1)  # [2560] flat [s, l]
        rcount = (1.0 / np.maximum(mask[c * SB : (c + 1) * SB].sum(1), 1.0))
        rcountb = np.ascontiguousarray(
            np.broadcast_to(rcount[None, :].astype(np.float16), (128, SB))
        )
        maskT = mask_c.reshape(L, 128).T.astype(np.float16).copy()
        sidT = (np.arange(T).reshape(L, 128).T // L).astype(np.float16)
        pidx_c = pos_idx[c * SB : (c + 1) * SB].reshape(-1).astype(np.int16)
        pidx16 = wrap16(pidx_c)  # [128, 160]
        # self/pair gather: idx = node >> 1 (fits int16), select bit node & 1
        xpair16 = wrap16((pos_nodes >> 1).astype(np.int16))  # [128, 168]
        bmask = np.ascontiguousarray(
            np.broadcast_to((pos_nodes & 1).astype(np.float16)[None, :], (128, NPOS))
        )

        cores.append(
            dict(
                idxA=idxA,
                idxB=idxB,
                dstlocAB=dstlocAB,
                invdegb=invdegb,
                maskT=np.ascontiguousarray(maskT),
                sidT=np.ascontiguousarray(sidT),
                rcountb=rcountb,
                pidx16=np.ascontiguousarray(pidx16),
                xpair16=np.ascontiguousarray(xpair16),
                bmask=bmask,
            )
        )
    return cores, CH


# --------------------------------------------------------------------------
# device kernel builder
# --------------------------------------------------------------------------


def _expand_mid(ap2d, count):
    """[128, F] -> [128, count, F] with stride-0 middle dim."""
    pdim = ap2d.ap[0]
    fdim = ap2d.ap[1]
    return bass.AP(ap2d.tensor, ap2d.offset, [pdim, [0, count], fdim])


def _expand_last(ap2d, count):
    """[128, K] -> [128, K, count] with stride-0 last dim."""
    pdim = ap2d.ap[0]
    kdim = ap2d.ap[1]
    return bass.AP(ap2d.tensor, ap2d.offset, [pdim, kdim, [0, count]])


def _build(CH, mock_cc=False):
    """Build the Bass program. mock_cc replaces the AllGather with local
    copies (for single-core simulators); results are then wrong for cores
    != 0 session blocks but timing/dependency shape is preserved."""
    CHA, CHB = CH
    CTA, CTB = int(np.sum(CHA)), int(np.sum(CHB))
    csA = np.concatenate([[0], np.cumsum(CHA)]).astype(np.int64)
    csB = np.concatenate([[0], np.cumsum(CHB)]).astype(np.int64)
    csAB = csA + csB
    SPLIT = 32768

    nc = bacc.Bacc(
        "TRN2", target_bir_lowering=False, debug=False, num_devices=NCORES
    )

    # ---- I/O ----
    def inp(name, shape, dt=F16):
        return nc.dram_tensor(name, list(shape), dt, kind="ExternalInput").ap()

    v2e = inp("v2e", [NN, EM])  # fp16 node table (edge gathers)
    v2ep = inp("v2ep", [NN // 2, 2 * EM])  # same data viewed as pairs
    v2eTs = inp("v2eTs", [EM, NSL])  # fp16 transposed slice (scoring)
    poswd = inp("poswd", [POSN, 2 * EM])  # pos table duplicated columns
    Wself = inp("Wself", [EM, EM])
    Wneigh = inp("Wneigh", [EM, EM])
    bhalf = inp("bhalf", [EM, 1], F32)  # 0.5 * b_sage
    w1a = inp("w1a", [EM, EM])
    w1b = inp("w1b", [EM, EM])
    glu1W = inp("glu1W", [EM, EM])
    glu2W = inp("glu2W", [EM, EM])
    glu3W = inp("glu3W", [EM, EM])
    glu4W = inp("glu4W", [EM, EM])
    w3 = inp("w3", [EM, EM])
    w2 = inp("w2", [EM, 1])
    w4 = inp("w4", [EM, 1])
    glu1b = inp("glu1b", [EM, 1], F32)
    glu3b = inp("glu3b", [EM, 1], F32)
    sc1 = inp("sc1", [EM, 1])
    sc2 = inp("sc2", [EM, 1])
    scb = inp("scb", [1, 1], F32)
    idxA_d = inp("idxA", [128, 8 * CTA], I16)
    idxB_d = inp("idxB", [128, 8 * max(CTB, 1)], I16)
    dstlocAB_d = inp("dstlocAB", [128, CTA + CTB])
    invdegb_d = inp("invdegb", [128, NPOS])
    rcountb_d = inp("rcountb", [128, SB])
    mexp_d = inp("mexp", [128, T])
    ident_d = inp("ident", [128, 128])
    iota_d = inp("iota16", [128, 128])
    maskT_d = inp("maskT", [128, L])
    sidT_d = inp("sidT", [128, L])
    pidx16_d = inp("pidx16", [128, T // 16], I16)
    xpair16_d = inp("xpair16", [128, NPOS // 16], I16)
    bmask_d = inp("bmask", [128, NPOS])

    scores = nc.dram_tensor("scores", [BS, NSL], F16, kind="ExternalOutput").ap()

    AF = mybir.ActivationFunctionType
    OP = mybir.AluOpType

    # user window FIRST (so q4 is ready early), then item windows in groups
    # of GW; the post-attention pipeline fires right after the last item
    # window with no user-dependent stragglers.
    groups = [[L]]
    w0 = 0
    while w0 < L:
        groups.append(list(range(w0, min(w0 + GW, L))))
        w0 += GW

    with tile.TileContext(nc) as tc:
        with (
            tc.tile_pool(name="const", bufs=1) as constp,
            tc.tile_pool(name="big", bufs=1) as bigp,
            tc.tile_pool(name="work", bufs=2) as workp,
            tc.tile_pool(name="psum", bufs=1, space="PSUM") as psp,
            tc.tile_pool(name="dram", bufs=1, space="DRAM") as dramp,
        ):
            def load(name, ap, shape, dt=F16):
                t = constp.tile(list(shape), dt, name=name + "_sb")
                nc.sync.dma_start(out=t, in_=ap)
                return t

            # ---------- edge gathers (idx tables loaded per group so the
            # first gather fires with minimal startup latency) ------------
            def gather_group(g):
                wlist = groups[g]
                a0, a1 = int(csA[wlist[0]]), int(csA[wlist[-1] + 1])
                b0, b1 = int(csB[wlist[0]]), int(csB[wlist[-1] + 1])
                gA = gB = None
                if a1 > a0:
                    ia = workp.tile([128, 8 * (a1 - a0)], I16, tag="idxAg",
                                    bufs=3)
                    nc.sync.dma_start(out=ia, in_=idxA_d[:, 8 * a0 : 8 * a1])
                    gA = workp.tile([128, (a1 - a0) * 128], F16, tag="gathA",
                                    bufs=3)
                    nc.gpsimd.dma_gather(
                        out_ap=gA[:, :].rearrange("p (c f) -> p c f", f=EM),
                        in_ap=v2e[:, :],
                        idxs_ap=ia[:, :],
                        num_idxs=(a1 - a0) * 128,
                        num_idxs_reg=(a1 - a0) * 128,
                        elem_size=EM,
                        single_packet=False,
                    )
                if b1 > b0:
                    ib = workp.tile([128, 8 * (b1 - b0)], I16, tag="idxBg",
                                    bufs=3)
                    nc.sync.dma_start(out=ib, in_=idxB_d[:, 8 * b0 : 8 * b1])
                    gB = workp.tile([128, (b1 - b0) * 128], F16, tag="gathB",
                                    bufs=3)
                    nc.gpsimd.dma_gather(
                        out_ap=gB[:, :].rearrange("p (c f) -> p c f", f=EM),
                        in_ap=v2e[SPLIT:, :],
                        idxs_ap=ib[:, :],
                        num_idxs=(b1 - b0) * 128,
                        num_idxs_reg=(b1 - b0) * 128,
                        elem_size=EM,
                        single_packet=False,
                    )
                return gA, gB, a0, b0

            gtiles = {0: gather_group(0)}

            # self-embedding pair gather (xT gates every window's h1)
            xpair16_t = load("xpair16_t", xpair16_d, [128, NPOS // 16], I16)
            xpairs = bigp.tile([128, 2 * NPOS], F16, name="xpairs")
            xpairs3 = xpairs.rearrange("p (j s) -> p j s", s=NPOS)
            nc.gpsimd.dma_gather(
                out_ap=xpairs3,
                in_ap=v2ep[:, :],
                idxs_ap=xpair16_t[:, :],
                num_idxs=NPOS,
                num_idxs_reg=NPOS,
                elem_size=2 * EM,
                transpose=True,
                single_packet=False,
            )

            gtiles[1] = gather_group(1)

            # position-embedding gather (needed first at window 3)
            pidx16_t = load("pidx16_t", pidx16_d, [128, T // 16], I16)
            posTd = bigp.tile([128, 2 * T], F16, name="posTd")
            posTd3 = posTd.rearrange("p (j t) -> p j t", t=T)
            nc.gpsimd.dma_gather(
                out_ap=posTd3,
                in_ap=poswd[:, :],
                idxs_ap=pidx16_t[:, :],
                num_idxs=T,
                num_idxs_reg=T,
                elem_size=2 * EM,
                transpose=True,
                single_packet=False,
            )
            posT = posTd3[:, 0, :]  # [f, t]

            # ---------- remaining constants (all host-shipped) ----------
            dstlocAB_t = load("dstlocAB_t", dstlocAB_d, [128, CTA + CTB])
            ident = load("ident", ident_d, [128, 128])
            iota16 = load("iota16", iota_d, [128, 128])
            ones_row = constp.tile([1, 128], F16)
            nc.vector.memset(ones_row, 1.0)
            # warm the activation-function tables off the critical path
            actwarm = constp.tile([1, 2], F16)
            nc.scalar.activation(actwarm, ones_row[:, 0:2], AF.Relu)
            nc.scalar.activation(actwarm, ones_row[:, 0:2], AF.Tanh)
            nc.scalar.activation(actwarm, ones_row[:, 0:2], AF.Sigmoid)

            wself_t = load("wself_t", Wself, [EM, EM])
            wneigh = load("wneigh", Wneigh, [EM, EM])
            bhalf_t = load("bhalf_t", bhalf, [EM, 1], F32)
            w1a_t = load("w1a_t", w1a, [EM, EM])
            w1b_t = load("w1b_t", w1b, [EM, EM])
            glu1w_t = load("glu1w_t", glu1W, [EM, EM])
            glu2w_t = load("glu2w_t", glu2W, [EM, EM])
            glu3w_t = load("glu3w_t", glu3W, [EM, EM])
            glu4w_t = load("glu4w_t", glu4W, [EM, EM])
            w3_t = load("w3_t", w3, [EM, EM])
            w2_t = load("w2_t", w2, [EM, 1])
            w4_t = load("w4_t", w4, [EM, 1])
            glu1b_t = load("glu1b_t", glu1b, [EM, 1], F32)
            glu3b_t = load("glu3b_t", glu3b, [EM, 1], F32)
            sc1_t = load("sc1_t", sc1, [EM, 1])
            sc2_t = load("sc2_t", sc2, [EM, 1])
            scb_t = load("scb_t", scb, [1, 1], F32)
            invdegb_t = load("invdegb_t", invdegb_d, [128, NPOS])
            rcountb_t = load("rcountb_t", rcountb_d, [128, SB])
            bmask_t = load("bmask_t", bmask_d, [128, NPOS])
            maskT_t = load("maskT_t", maskT_d, [128, L])
            sidT_t = load("sidT_t", sidT_d, [128, L])

            # ---------- big persistent tiles ----------
            xT = bigp.tile([128, NPOS], F16, name="xT")  # [f, slot] self emb
            ne_T = bigp.tile([128, NPOS], F16, name="ne_T")  # [f, slot]
            ne_tf = bigp.tile([128, L * 128], F16, name="ne_tf")  # [t, f]
            mm_t = bigp.tile([128, L * 128], F16, name="mm_t")
            nhT = bigp.tile([128, T], F16, name="nhT")
            nh2fT = bigp.tile([128, T], F16, name="nh2fT")
            nh1T = bigp.tile([128, T], F16, name="nh1T")
            nh2T = bigp.tile([128, T], F16, name="nh2T")

            netf3 = ne_tf.rearrange("p (w q) -> p w q", q=128)

            # xT = lo + (hi - lo) * b
            xdiff = bigp.tile([128, NPOS], F16, name="xdiff")
            nc.vector.tensor_sub(xdiff, xpairs3[:, 1, :], xpairs3[:, 0, :])
            nc.vector.tensor_tensor(
                out=xdiff, in0=xdiff, in1=bmask_t, op=OP.mult
            )
            nc.vector.tensor_add(xT, xpairs3[:, 0, :], xdiff)

            # tmp accumulation psum in [f, s] layout: sum_t ne[f,t]*m[t,s]
            ps_tmp = psp.tile([128, 128], F32, tag="acc", bufs=2, space="PSUM")
            ps_su = psp.tile([128, 128], F32, tag="acc", bufs=2, space="PSUM")
            q4 = None  # assigned at the user window (processed first)

            # attention stages that need only ne_T/posT (tanh halves),
            # interleaved into phase 1 as windows complete
            def att_pre(i):
                sl = slice(i * ATT_CHUNK, (i + 1) * ATT_CHUNK)
                ps_a = psp.tile([128, ATT_CHUNK], F32, tag="mm", bufs=4,
                                space="PSUM")
                nc.tensor.matmul(ps_a, lhsT=w1a_t, rhs=posT[:, sl],
                                 start=True, stop=False)
                nc.tensor.matmul(ps_a, lhsT=w1b_t, rhs=ne_T[:, sl],
                                 start=False, stop=True)
                nc.scalar.activation(nh1T[:, sl], ps_a, AF.Tanh)
                ps_c = psp.tile([128, ATT_CHUNK], F32, tag="mm", bufs=4,
                                space="PSUM")
                nc.tensor.matmul(ps_c, lhsT=w3_t, rhs=ne_T[:, sl],
                                 start=True, stop=True)
                nc.scalar.activation(nh2T[:, sl], ps_c, AF.Tanh)

            ps_sv = None

            # ---------- phase 2/3, q2-dependent half (after last item win)
            def emit_post_item():
                nonlocal ps_sv
                # hs^T directly from the [f, s] tmp accumulation
                hsT = workp.tile([128, 128], F16, tag="hsT", bufs=1)
                nc.vector.tensor_tensor(
                    out=hsT, in0=ps_tmp, in1=rcountb_t, op=OP.mult
                )
                ps_q = psp.tile([128, 128], F32, tag="mm", bufs=4,
                                space="PSUM")
                nc.tensor.matmul(ps_q, lhsT=hsT, rhs=glu2w_t, start=True,
                                 stop=True)
                q2 = workp.tile([128, 128], F16, tag="q2", bufs=1)
                nc.vector.tensor_copy(q2, ps_q)

                ps_sv = psp.tile([128, 128], F32, tag="acc", bufs=2,
                                 space="PSUM")
                BCH = 512
                WPC_ = BCH // 128
                for i in range(T // BCH):
                    sl = slice(i * BCH, (i + 1) * BCH)
                    ps_b = psp.tile([128, BCH], F32, tag="mm", bufs=4,
                                    space="PSUM")
                    nc.tensor.matmul(ps_b, lhsT=glu1w_t, rhs=nh1T[:, sl],
                                     start=True, stop=False)
                    nc.tensor.matmul(ps_b, lhsT=q2, rhs=mexp[:, sl],
                                     start=False, stop=True)
                    nc.scalar.activation(nhT[:, sl], ps_b, AF.Sigmoid,
                                         bias=glu1b_t[:, :])
                    ps_d = psp.tile([128, BCH], F32, tag="mm", bufs=4,
                                    space="PSUM")
                    nc.tensor.matmul(ps_d, lhsT=glu3w_t, rhs=nh2T[:, sl],
                                     start=True, stop=False)
                    nc.tensor.matmul(ps_d, lhsT=q4, rhs=mexp[:, sl],
                                     start=False, stop=True)
                    nc.scalar.activation(nh2fT[:, sl], ps_d, AF.Sigmoid,
                                         bias=glu3b_t[:, :])
                    # batched: all beta matmuls, then all ms, then accums —
                    # keeps the PE accumulation chain streaming
                    wlo, whi = i * WPC_, (i + 1) * WPC_
                    ps_bb = psp.tile([128, 2 * WPC_], F32, tag="mm", bufs=4,
                                     space="PSUM")
                    for w in range(wlo, whi):
                        wsl = slice(w * 128, (w + 1) * 128)
                        j = 2 * (w - wlo)
                        nc.tensor.matmul(ps_bb[:, j : j + 1],
                                         lhsT=nhT[:, wsl], rhs=w2_t,
                                         start=True, stop=True)
                        nc.tensor.matmul(ps_bb[:, j + 1 : j + 2],
                                         lhsT=nh2fT[:, wsl], rhs=w4_t,
                                         start=True, stop=True)
                    mss = []
                    for w in range(wlo, whi):
                        wsl = slice(w * 128, (w + 1) * 128)
                        j = 2 * (w - wlo)
                        ms1 = workp.tile([128, 128], F16, tag="ms", bufs=8)
                        nc.vector.tensor_scalar_mul(
                            ms1, mm_t[:, wsl], ps_bb[:, j : j + 1]
                        )
                        ms2 = workp.tile([128, 128], F16, tag="ms2", bufs=8)
                        nc.vector.tensor_scalar_mul(
                            ms2, mm_t[:, wsl], ps_bb[:, j + 1 : j + 2]
                        )
                        mss.append((ms1, ms2))
                    for w in range(wlo, whi):
                        ms1, ms2 = mss[w - wlo]
                        nc.tensor.matmul(
                            ps_sv, lhsT=ms1, rhs=netf3[:, w, :],
                            start=(w == 0), stop=(w == L - 1),
                        )
                        nc.tensor.matmul(
                            ps_su, lhsT=ms2, rhs=netf3[:, w, :],
                            start=(w == 0), stop=(w == L - 1),
                        )


            # ---------- phase 1: per-window SAGE ----------
            WPC = ATT_CHUNK // 128  # windows per attention chunk
            mexp = None
            for g in range(len(groups)):
                if g + 2 < len(groups):
                    gtiles[g + 2] = gather_group(g + 2)
                if g == len(groups) - 1:
                    # mexp built on Pool after the last gather prep (its
                    # queue is idle then; zero DMA traffic)
                    mexp = constp.tile([128, T], F16, name="mexp_sb")
                    nc.gpsimd.memset(mexp, 0.0)
                    nc.gpsimd.affine_select(
                        out=mexp, in_=mexp, compare_op=OP.not_equal,
                        fill=1.0, base=0, pattern=[[1, SB], [0, L]],
                        channel_multiplier=-1,
                    )
                gA, gB, ga0, gb0 = gtiles.pop(g)
                for w in groups[g]:
                    wsl = slice(w * 128, (w + 1) * 128)
                    nch_tot = CHA[w] + CHB[w]

                    # one one-hot build for the window's A+B chunks
                    s_t = workp.tile([128, nch_tot * 128], F16, tag="onehot",
                                     bufs=3)
                    s3 = s_t.rearrange("p (c f) -> p c f", f=128)
                    ab0 = int(csAB[w])
                    nc.vector.tensor_tensor(
                        out=s3,
                        in0=_expand_mid(iota16[:, :], nch_tot),
                        in1=_expand_last(
                            dstlocAB_t[:, ab0 : ab0 + nch_tot], 128
                        ),
                        op=OP.is_equal,
                    )
                    parts = []
                    if CHA[w] > 0:
                        parts.append((gA, int(csA[w]) - ga0, 0, CHA[w]))
                    if CHB[w] > 0:
                        parts.append((gB, int(csB[w]) - gb0, CHA[w], CHB[w]))

                    ps_agg = psp.tile([128, 128], F32, tag="mm", bufs=4,
                                      space="PSUM")
                    kdone = 0
                    for gt, goff, soff, chn in parts:
                        for c in range(chn):
                            gsl = slice((goff + c) * 128, (goff + c + 1) * 128)
                            csl = slice((soff + c) * 128, (soff + c + 1) * 128)
                            nc.tensor.matmul(
                                ps_agg,
                                lhsT=gt[:, gsl],
                                rhs=s_t[:, csl],
                                start=(kdone == 0),
                                stop=(kdone == nch_tot - 1),
                            )
                            kdone += 1
                    # mean -> fp16
                    hn = workp.tile([128, 128], F16, tag="hn", bufs=2)
                    nc.vector.tensor_tensor(
                        out=hn, in0=ps_agg, in1=invdegb_t[:, wsl], op=OP.mult
                    )
                    # h1_half = relu(0.5*(x@Wself + hn@Wneigh + b))
                    ps_h1 = psp.tile([128, 128], F32, tag="mm", bufs=4,
                                     space="PSUM")
                    nc.tensor.matmul(ps_h1, lhsT=wself_t, rhs=xT[:, wsl],
                                     start=True, stop=False)
                    nc.tensor.matmul(ps_h1, lhsT=wneigh, rhs=hn,
                                     start=False, stop=True)
                    h1h = workp.tile([128, 128], F16, tag="h1h", bufs=2)
                    nc.scalar.activation(
                        h1h, ps_h1, AF.Relu, bias=bhalf_t[:, :], scale=0.5
                    )
                    # ne = 0.5*h1 + 0.5*x = h1h + 0.5*xT
                    nc.vector.scalar_tensor_tensor(
                        out=ne_T[:, wsl], in0=xT[:, wsl], scalar=0.5,
                        in1=h1h, op0=OP.mult, op1=OP.add,
                    )

                    if w == L:
                        # user window (processed first): q4 = user_emb @ glu4W
                        ps_q4 = psp.tile([128, 128], F32, tag="mm", bufs=4,
                                         space="PSUM")
                        nc.tensor.matmul(ps_q4, lhsT=ne_T[:, T:NPOS],
                                         rhs=glu4w_t, start=True, stop=True)
                        q4 = workp.tile([128, 128], F16, tag="q4", bufs=1)
                        nc.vector.tensor_copy(q4, ps_q4)
                    else:
                        ps_tf = psp.tile([128, 128], F16, tag="mmT", bufs=2,
                                         space="PSUM")
                        nc.tensor.transpose(ps_tf, ne_T[:, wsl], ident)
                        if w % 2 == 0:
                            nc.vector.tensor_copy(netf3[:, w, :], ps_tf)
                        else:
                            nc.scalar.copy(netf3[:, w, :], ps_tf)
                        nc.vector.scalar_tensor_tensor(
                            out=mm_t[:, wsl],
                            in0=iota16,
                            scalar=sidT_t[:, w : w + 1],
                            in1=maskT_t[:, w : w + 1].to_broadcast([128, 128]),
                            op0=OP.is_equal,
                            op1=OP.mult,
                        )
                        nc.tensor.matmul(
                            ps_tmp,
                            lhsT=netf3[:, w, :],
                            rhs=mm_t[:, wsl],
                            start=(w == 0),
                            stop=(w == L - 1),
                        )
                        if (w + 1) % WPC == 0:
                            att_pre(w // WPC)
                        if w == L - 1:
                            emit_post_item()

            user_embT = ne_T[:, T:NPOS]  # [f, s]

            # ---------- phase 4: combine ----------
            sv_sb = workp.tile([128, 128], F16, tag="sv_sb", bufs=1)
            nc.vector.tensor_copy(sv_sb, ps_sv)
            su_sb = workp.tile([128, 128], F16, tag="su_sb", bufs=1)
            nc.scalar.copy(su_sb, ps_su)
            ps_t2 = psp.tile([128, 128], F16, tag="mmT", bufs=2, space="PSUM")
            nc.tensor.transpose(ps_t2, sv_sb, ident)
            svT = workp.tile([128, 128], F16, tag="svT", bufs=1)
            nc.vector.tensor_copy(svT, ps_t2)
            ps_t3 = psp.tile([128, 128], F16, tag="mmT", bufs=2, space="PSUM")
            nc.tensor.transpose(ps_t3, su_sb, ident)
            suT = workp.tile([128, 128], F16, tag="suT", bufs=1)
            nc.vector.tensor_copy(suT, ps_t3)

            ps_al = psp.tile([1, 128], F32, tag="mm", bufs=4, space="PSUM")
            nc.tensor.matmul(ps_al, lhsT=sc1_t, rhs=svT, start=True, stop=False)
            nc.tensor.matmul(ps_al, lhsT=sc2_t, rhs=suT, start=False, stop=True)
            alphaT = workp.tile([1, 128], F16, tag="alphaT", bufs=1)
            nc.scalar.activation(alphaT, ps_al, AF.Sigmoid, bias=scb_t[:, :])
            ps_ab = psp.tile([128, 128], F32, tag="mm", bufs=4, space="PSUM")
            nc.tensor.matmul(ps_ab, lhsT=ones_row, rhs=alphaT, start=True,
                             stop=True)
            ab_sb = workp.tile([128, 128], F16, tag="ab_sb", bufs=1)
            nc.scalar.copy(ab_sb, ps_ab)

            dvT = workp.tile([128, 128], F16, tag="dvT", bufs=1)
            nc.vector.tensor_sub(dvT, svT, suT)
            adT = workp.tile([128, 128], F16, tag="adT", bufs=1)
            nc.vector.tensor_tensor(out=adT, in0=dvT, in1=ab_sb, op=OP.mult)
            t1T = workp.tile([128, 128], F16, tag="t1T", bufs=1)
            nc.vector.tensor_add(t1T, user_embT, suT)
            seqT = workp.tile([128, 128], F16, tag="seqT", bufs=1)
            nc.vector.tensor_add(seqT, t1T, adT)

            # ---------- phase 4b: AllGather session vectors ----------
            cc_in = dramp.tile([128, 128], F16)
            cc_out = dramp.tile([NCORES * 128, 128], F16)
            nc.sync.dma_start(out=cc_in[:, :], in_=seqT[:, :])
            if mock_cc:
                for b in range(NCORES):
                    nc.sync.dma_start(
                        out=cc_out[b * 128 : (b + 1) * 128, :], in_=cc_in[:, :]
                    )
            else:
                nc.gpsimd.collective_compute(
                    "AllGather",
                    mybir.AluOpType.bypass,
                    replica_groups=[list(range(NCORES))],
                    ins=[cc_in[:, :]],
                    outs=[cc_out[:, :]],
                )
            # one strided copy: cc_out [(b f), s] -> seqg [f, (b s)]
            seqg = workp.tile([128, NCORES * 128], F16, tag="seqg", bufs=1)
            cc_ap = cc_out[:, :]
            cc_src = bass.AP(
                cc_ap.tensor,
                cc_ap.offset,
                [[128, 128], [128 * 128, NCORES], [1, 128]],
            )
            nc.sync.dma_start(
                out=seqg[:, :].rearrange("p (b s) -> p b s", s=128),
                in_=cc_src,
            )

            # ---------- phase 5: scoring (item-sharded) ----------
            # item-chunk outer, session-block inner; the v2eT slice streams
            # in during the scoring phase (overlapped with score output)
            sizes = [512]
            off = 512
            while off < NSL:
                sizes.append(min(SCORE_CHUNK, NSL - off))
                off += sizes[-1]
            # first two input chunks issued from the Act queue: it drains
            # right as the post-attention pipeline ends, landing the loads
            # in the pre-scoring DMA idle window
            vts = []
            off = 0
            for j, sz in enumerate(sizes):
                vt = workp.tile([128, SCORE_CHUNK], F16, tag="vstream",
                                bufs=3)
                if j < 2:
                    nc.scalar.dma_start(out=vt[:, :sz],
                                        in_=v2eTs[:, off : off + sz])
                vts.append(vt)
                off += sz
            # keep the PE clock ramped through the collective wait: dummy
            # matmuls on the first streamed chunk (results never read)
            for wk in range(24):
                ps_w = psp.tile([128, 512], F32, tag="mmT", bufs=2,
                                space="PSUM")
                nc.tensor.matmul(
                    ps_w[:, 0:256], lhsT=wself_t, rhs=vts[0][:, 0:256],
                    start=True, stop=True,
                )

            ev = 0
            off = 0
            for j, sz in enumerate(sizes):
                vt = vts[j]
                if j >= 2:
                    nc.sync.dma_start(out=vt[:, :sz],
                                      in_=v2eTs[:, off : off + sz])
                for b in range(NCORES):
                    ob = workp.tile([128, SCORE_CHUNK], F16, tag="obuf",
                                    bufs=6)
                    k0 = 0
                    while k0 < sz:
                        kw = min(512, sz - k0)
                        ps_s = psp.tile([128, 512], F32, tag="mm", bufs=4,
                                        space="PSUM")
                        nc.tensor.matmul(
                            ps_s[:, :kw],
                            lhsT=seqg[:, b * 128 : (b + 1) * 128],
                            rhs=vt[:, k0 : k0 + kw],
                            start=True, stop=True,
                        )
                        if ev % 2 == 0:
                            nc.vector.tensor_copy(ob[:, k0 : k0 + kw],
                                                  ps_s[:, :kw])
                        else:
                            nc.scalar.copy(ob[:, k0 : k0 + kw], ps_s[:, :kw])
                        ev += 1
                        k0 += kw
                    nc.sync.dma_start(
                        out=scores[b * 128 : (b + 1) * 128, off : off + sz],
                        in_=ob[:, :sz],
                    )
                off += sz

    nc.compile()
    return nc


# --------------------------------------------------------------------------
# entry point
# --------------------------------------------------------------------------


def _make_in_maps(inputs, cores_pre):
    v2e32 = np.asarray(inputs["v2e_weight"], np.float32)
    v2e16 = np.ascontiguousarray(v2e32.astype(np.float16))
    v2eT16 = np.ascontiguousarray(v2e32.T.astype(np.float16))
    posw = np.asarray(inputs["pos_weight"], np.float32).astype(np.float16)
    poswd = np.ascontiguousarray(np.concatenate([posw, posw], axis=1))
    f16 = lambda a: np.ascontiguousarray(np.asarray(a, np.float32).astype(np.float16))
    f32c = lambda a: np.ascontiguousarray(np.asarray(a, np.float32))

    W_self = np.asarray(inputs["W_self"], np.float32)
    W_neigh = np.asarray(inputs["W_neigh"], np.float32)
    b_sage = np.asarray(inputs["b_sage"], np.float32)
    w_1 = np.asarray(inputs["w_1"], np.float32)
    sc_W = np.asarray(inputs["sc_W"], np.float32)

    shared = dict(
        v2e=v2e16,
        v2ep=np.ascontiguousarray(v2e16.reshape(NN // 2, 2 * EM)),
        poswd=poswd,
        Wself=f16(W_self),
        Wneigh=f16(W_neigh),
        bhalf=f32c(0.5 * b_sage.reshape(EM, 1)),
        w1a=f16(w_1[:EM]),
        w1b=f16(w_1[EM:]),
        glu1W=f16(inputs["glu1_W"]),
        glu2W=f16(inputs["glu2_W"]),
        glu3W=f16(inputs["glu3_W"]),
        glu4W=f16(inputs["glu4_W"]),
        w3=f16(inputs["w_3"]),
        w2=f16(np.asarray(inputs["w_2"], np.float32).reshape(EM, 1)),
        w4=f16(np.asarray(inputs["w_4"], np.float32).reshape(EM, 1)),
        glu1b=f32c(np.asarray(inputs["glu1_b"], np.float32).reshape(EM, 1)),
        glu3b=f32c(np.asarray(inputs["glu3_b"], np.float32).reshape(EM, 1)),
        sc1=f16(sc_W[:EM].reshape(EM, 1)),
        sc2=f16(sc_W[EM:].reshape(EM, 1)),
        scb=f32c(np.asarray(inputs["sc_b"], np.float32).reshape(1, 1)),
        mexp=np.ascontiguousarray(
            (np.arange(T)[None, :] // L == np.arange(128)[:, None])
            .astype(np.float16)
        ),
        ident=np.eye(128, dtype=np.float16),
        iota16=np.ascontiguousarray(
            np.broadcast_to(np.arange(128, dtype=np.float16)[None, :],
                            (128, 128))
        ),
    )
    in_maps = []
    for c in range(NCORES):
        m = dict(shared)
        m["v2eTs"] = np.ascontiguousarray(v2eT16[:, c * NSL : (c + 1) * NSL])
        m.update(cores_pre[c])
        in_maps.append(m)
    return in_maps


def kernel(**inputs) -> np.ndarray:
    cores_pre, CH = _preprocess(
        inputs["src"], inputs["dst"], inputs["user"], inputs["seq"],
        inputs["mask"], inputs["pos_idx"],
    )
    key = CH
    if key not in _CACHE:
        _CACHE[key] = _build(CH)
    nc = _CACHE[key]
    in_maps = _make_in_maps(inputs, cores_pre)
    res = run_bass_kernel_spmd(nc, in_maps, core_ids=list(range(NCORES)))
    out = np.empty((BS, NN), np.float32)
    for c in range(NCORES):
        out[:, c * NSL : (c + 1) * NSL] = res.results[c]["scores"].astype(
            np.float32
        )
    return out


# expose for test harness
def build_and_inputs(inputs, mock_cc=False):
    cores_pre, CH = _preprocess(
        inputs["src"], inputs["dst"], inputs["user"], inputs["seq"],
        inputs["mask"], inputs["pos_idx"],
    )
    nc = _build(CH, mock_cc=mock_cc)
    in_maps = _make_in_maps(inputs, cores_pre)
    return nc, in_maps
